# revision 62
# baseline (speedup 1.0000x reference)
"""Trainium2 Bass kernel for nn_AttentionProbe_80891414053184.

Math (reference):
    y  = relu(x @ W1.T + b1)            # (B,S,H) -> (B,S,128)
    y2 = relu(y @ W2.T + b2)            # (B,S,128)
    l  = y2 @ Wq.T + pos*pos_w  (+mask) # (B,S,8) logits
    p  = softmax(l, axis=S)
    v  = y2 @ Wv.T + bv
    out[b] = sum_{s,h} p*v + bias       # (B,1)

Strategy: sequence-parallel over 8 cores (512 positions x 4 batches = 2048
tokens per core).  Each core streams its x-shard quantized to fp8-e4m3
(half the HBM bytes of bf16; end-to-end rel-err vs the fp32 reference
~5e-3, HW-measured) CHUNK-MAJOR across all 4 token tiles, layer 1 as
DoubleRow fp8 matmuls (256-deep contraction per instruction).  Chunk-major
keeps PE duty during the stream at ~60%, under the package power-throttle
threshold -- tile-major variants that fill the PE get DVFS-clamped to 50%
and lose more than they gain (HW-measured).  The last two chunks land
per-tile so each tile's MLP tail + softmax stats start as soon as its
accumulation closes.  Per-tile partial softmax stats (-max, Z, W) are
emitted per (seq-quarter, head) lane; the host merges 8 cores x 4
quarters with the standard online-softmax combine into the (4,1) output.

fp8 scaling: W1 is pre-scaled by 64 on the host so its N(0, 1/4096)
entries land in e4m3's normal range (min normal 2^-6); the 64x is folded
back via b1*64 at the relu (relu commutes with positive scale) and W2/64
in the layer-2 weights.  x itself is N(0,1) -- quantized unscaled.

Stats packing: per tile, q and v head projections land in one (128, 256)
psum (q cols 0:128, v cols 128:256), one 8-lane matmul per seq quarter at
partition offset 32*qq via tile_position.  The softmax-stats chain then
runs on 128-column DVE/ACT ops (~3x shorter than a 512-column chain).
bf16 is used for y2 and the head weights: the 128-col moving operand
would hit f32r's 4x penalty below 256 columns.
"""

import numpy as np

# Problem dims (hardcoded per harness contract).
B, S, H = 4, 4096, 4096
MLP, NH = 128, 8
NCORES = 8
S_SHARD = S // NCORES        # 512 seq positions per core
TOK = B * S_SHARD            # 2048 tokens per core
NT = TOK // 512              # 4 token tiles of 512 (= one batch each)
NQ = 4                       # seq quarters per tile (128 cols each)
KC2 = H // 256               # 16 double-chunks (256-deep DoubleRow contraction)
GRP = 2                      # double-chunks per streaming x DMA (1 MB fp8)
NGRP = KC2 // GRP            # 8 groups; the last arrives per (tile, chunk)
W1SCALE = 64.0               # fp8 pre-scale for W1 (power of 2, exact)

_cache = {}


def _build_nc(affine):
    """affine=True: the additive logit (pos_w*pos, mask all-ones) is
    generated on-chip by a rank-2 matmul pre-filling the q/v psum -- saves
    the 262 KB ca table from the HBM stream and two DVE chain steps.
    affine=False: general-mask path, ca table streamed from HBM."""
    import concourse.mybir as mybir
    import concourse.tile as tile
    from concourse import bacc

    f32 = mybir.dt.float32
    f32r = mybir.dt.float32r
    fp8 = mybir.dt.float8e4
    bf16 = mybir.dt.bfloat16
    DR = mybir.MatmulPerfMode.DoubleRow

    # Bacc (not bare Bass): its finalize() runs move_matmul_waits_to_ldweights
    # and generate_event_semaphores, which split multi-sem waits to satisfy
    # TRN2's one-wait-per-instruction encoding limit.
    nc = bacc.Bacc()
    # xt row r = c*128 + p holds [j=0 | j=1] token rows for feature
    # f = c*256 + j*128 + p -- the DoubleRow kxn layout with the two k-tile
    # planes exactly 2048 B apart in SBUF (other spacings hit SBUF
    # conflicts that halve the double-pumped PE read rate, HW-measured).
    xt_d = nc.dram_tensor("xt", [KC2 * 128, 2, TOK], fp8,
                          kind="ExternalInput")
    # w1s[p, c, j, m] = 64 * W1[m, c*256 + j*128 + p]
    w1_d = nc.dram_tensor("w1s", [128, KC2, 2, MLP], fp8, kind="ExternalInput")
    # cwr: W2.T/64 f32r -- the layer-2 matmul at the fast PE rate
    cwr_d = nc.dram_tensor("cwr", [MLP, MLP], f32r, kind="ExternalInput")
    # chd: [Wq.T | Wv.T] bf16
    chd_d = nc.dram_tensor("chd", [MLP, 2 * NH], bf16, kind="ExternalInput")
    # cw: [64*b1 | b2] per-partition scalar columns
    cw_d = nc.dram_tensor("cw", [MLP, 2], f32, kind="ExternalInput")
    if affine:
        # cab row 0/1 = (A, B) coefficient column blocks: the additive
        # logit A+B*n on lanes 32*qq+h (identical for every tile -- mask is
        # all ones), then a bv block (B=0), then the ramp block (row0 =
        # ones, row1 = iota) used as the moving operand.
        CBV = 128                    # bv block col offset
        CM16 = 256                   # constant -16 block (exp offset)
        CRMP = 384                   # ramp block col offset
        cab_d = nc.dram_tensor("cab", [2, CRMP + 128], f32,
                               kind="ExternalInput")
    else:
        # ca: per tile a (128, 128) additive-logit block (pos_w*pos + mask)
        # on lanes 32*qq+h; final col = bv on the same lanes.
        ca_d = nc.dram_tensor("ca", [128, NT * 128 + 1], f32,
                              kind="ExternalInput")
    # stats: per tile 3 cols [-m | Z | W]; lanes 32*qq+h are valid.
    st_d = nc.dram_tensor("stats", [128, NT * 3], f32, kind="ExternalOutput")

    AF = mybir.ActivationFunctionType
    AX = mybir.AxisListType
    OP = mybir.AluOpType

    with tile.TileContext(nc) as tc:
        with (
            tc.tile_pool(name="const", bufs=1) as const,
            tc.tile_pool(name="yp", bufs=2) as yp,
            tc.tile_pool(name="y2p", bufs=2) as y2p,
            tc.tile_pool(name="smallp", bufs=2) as smallp,
            tc.tile_pool(name="statsp", bufs=1) as statsp,
            tc.tile_pool(name="ps_y", bufs=4, space="PSUM") as ps_y,
            tc.tile_pool(name="ps_y2", bufs=2, space="PSUM") as ps_y2,
            tc.tile_pool(name="ps_qv", bufs=2, space="PSUM") as ps_qv,
        ):
            # The full fp8 x-shard lives in SBUF (64 KB/partition): no slot
            # recycling, so the stream DMAs carry no WAR deps on the PE and
            # need no escort ops.  Issue every x DMA up front on the gpsimd
            # (SWDGE) queue; Q7 descriptor emission (~1.2 us each) stays
            # ahead of the ~3 us per-group transfer time.
            x_sb = const.tile([128, KC2, 2, TOK], fp8)
            # Group 0 rides the sync (HWDGE) ring: it starts right at the
            # init-barrier exit (~0.9 us before the gpsimd Q7 wakes), and
            # the SDMA engines drain both rings concurrently, so the whole
            # stream finishes earlier.
            for g in range(NGRP - 1):
                eng = nc.sync if g == 0 else nc.gpsimd
                eng.dma_start(
                    out=x_sb[:, g * GRP:(g + 1) * GRP, :, :],
                    in_=xt_d[g * GRP * 128:(g + 1) * GRP * 128, :, :].rearrange(
                        "(a p) j n -> p a j n", p=128))
            # Chunk 14 for all tiles rides one half-group; then a single
            # 128 KB chunk-15 closer per tile, so tile t's accumulation
            # closes (and its tail starts) with minimal end-of-stream
            # serialization.
            nc.gpsimd.dma_start(
                out=x_sb[:, 14:15, :, :],
                in_=xt_d[14 * 128:15 * 128, :, :].rearrange(
                    "(a p) j n -> p a j n", p=128))
            for t in range(NT):
                # Tile 3's closer rides the (empty-by-then) sync ring so it
                # drains concurrently with the gpsimd ring's c14 group and
                # t0-2 closers -- the terminal tile closes ~1.5 us earlier.
                eng = nc.sync if t == NT - 1 else nc.gpsimd
                eng.dma_start(
                    out=x_sb[:, 15, :, t * 512:(t + 1) * 512],
                    in_=xt_d[15 * 128:16 * 128, :,
                             t * 512:(t + 1) * 512].rearrange(
                                 "(p) j n -> p j n", p=128))

            # Const loads on the sync (HWDGE) queue -- separate path from the
            # x stream.  w1 chunk 0 is split out so the first matmul gates on
            # a 32 KB transfer, not the full 512 KB.
            w1_sb = const.tile([128, KC2, 2, MLP], fp8)
            nc.sync.dma_start(out=w1_sb[:, 0:1, :, :], in_=w1_d[:, 0:1, :, :])
            nc.sync.dma_start(out=w1_sb[:, 1:KC2, :, :], in_=w1_d[:, 1:KC2, :, :])
            if affine:
                cab_sb = const.tile([2, CRMP + 128], f32)
                nc.sync.dma_start(out=cab_sb[:], in_=cab_d[:])
            else:
                ca_sb = const.tile([128, NT * 128 + 1], f32)
                nc.sync.dma_start(out=ca_sb[:], in_=ca_d[:])
            cw_sb = const.tile([MLP, 2], f32)
            nc.sync.dma_start(out=cw_sb[:], in_=cw_d[:])
            cwr_sb = const.tile([MLP, MLP], f32r)
            nc.sync.dma_start(out=cwr_sb[:], in_=cwr_d[:])
            chd_sb = const.tile([MLP, 2 * NH], bf16)
            nc.sync.dma_start(out=chd_sb[:], in_=chd_d[:])

            stats_sb = statsp.tile([128, NT * 3], f32)

            # --- Warmup / staging: each engine observes every const-DMA lane
            # once, so steady-state instructions carry at most one new wait
            # (fewer split-events from Bacc's generate_event_semaphores).
            # Only the w1 warmup gates the k-loop; the other warms run later
            # so the first real matmul waits on nothing but w1-chunk0 +
            # x-group0.
            warm_ps = ps_y2.tile([128, 512], f32, tag="y2", name="warm_ps")
            warm_pe_last = nc.tensor.matmul(warm_ps[:, 0:NH],
                                            w1_sb[:, 0, :, :],
                                            w1_sb[:, 0, :, 0:NH],
                                            start=True, stop=True,
                                            perf_mode=DR)

            # Layer 1, chunk-major over the streaming groups: yT[t]
            # (128, 512) += (64*W1T)_c.T @ xT_c, DoubleRow fp8 (256-deep
            # contraction per matmul), k-accumulated over 16 double-chunks.
            psum_y = [ps_y.tile([128, 512], f32, tag="y", name=f"y_ps{t}")
                      for t in range(NT)]
            first = True
            for c in range(GRP * (NGRP - 1)):
                for t in range(NT):
                    mm = nc.tensor.matmul(
                        psum_y[t][:],
                        w1_sb[:, c, :, :],
                        x_sb[:, c, :, t * 512:(t + 1) * 512],
                        start=(c == 0),
                        stop=False,
                        perf_mode=DR,
                    )
                    if first:
                        first = False
                        tile.add_dep_helper(mm.ins, warm_pe_last.ins,
                                            sync=False,
                                            reason="warmups before first mm")

            # cw/ca/cwr/chd lane warmups (before their first consumers in
            # tile 0's tail)
            warm_ps2 = ps_y2.tile([128, 512], f32, tag="y2", name="warm_ps2")
            nc.tensor.matmul(warm_ps2[0:NH, 0:NH], cwr_sb[:, 0:NH],
                             cwr_sb[:, 0:NH], start=True, stop=True)
            nc.tensor.matmul(warm_ps2[0:2 * NH, NH:2 * NH], chd_sb[:],
                             chd_sb[:, 0:NH], start=True, stop=True)
            warm_act = const.tile([MLP, 1], f32)
            nc.scalar.copy(out=warm_act[:], in_=cw_sb[:, 1:2])
            warm_dve = const.tile([128, 1], f32)
            if affine:
                # Build the (tile-invariant) additive-logit block A + B*n
                # and the bv column once, on-chip: two rank-2 matmuls into
                # psum, one DVE copy to SBUF.  Replaces the 262 KB ca table
                # in the HBM stream.
                ab_ps = ps_y2.tile([128, 512], f32, tag="y2", name="ab_ps")
                nc.tensor.matmul(ab_ps[:, 0:128], cab_sb[:, 0:128],
                                 cab_sb[:, CRMP:CRMP + 128],
                                 start=True, stop=True)
                nc.tensor.matmul(ab_ps[:, 128:129],
                                 cab_sb[:, CBV:CBV + 128],
                                 cab_sb[:, CRMP:CRMP + 1],
                                 start=True, stop=True)
                nc.tensor.matmul(ab_ps[:, 129:130],
                                 cab_sb[:, CM16:CM16 + 128],
                                 cab_sb[:, CRMP:CRMP + 1],
                                 start=True, stop=True)
                ca_blk = const.tile([128, 130], f32)
                nc.vector.tensor_copy(out=ca_blk[:], in_=ab_ps[:, 0:130])
                nc.vector.tensor_copy(out=warm_dve[:], in_=cw_sb[:, 0:1])
            else:
                warm_act8 = const.tile([128, 1], f32)
                nc.scalar.copy(out=warm_act8[:],
                               in_=ca_sb[:, NT * 128:NT * 128 + 1])
                nc.vector.tensor_copy(out=warm_dve[:], in_=ca_sb[:, 0:1])

            # Per tile: close the accumulation with its own last chunks,
            # then run the MLP tail + softmax stats for that tile while the
            # next tile's last chunks land.
            for t in range(NT):
                for c in range(GRP * (NGRP - 1), KC2):
                    nc.tensor.matmul(
                        psum_y[t][:],
                        w1_sb[:, c, :, :],
                        x_sb[:, c, :, t * 512:(t + 1) * 512],
                        start=False,
                        stop=(c == KC2 - 1),
                        perf_mode=DR,
                    )

                y_sb = yp.tile([128, 512], f32r, tag="ysb", name=f"y_sb{t}")
                y2_ps = ps_y2.tile([128, 512], f32, tag="y2", name=f"y2_ps{t}")
                y2_sb = y2p.tile([128, 512], bf16, tag="y2sb",
                                 name=f"y2_sb{t}")
                # relu on DVE (add+max) keeps ACT free for relu2/exp.  psum
                # holds 64*(x@W1.T); +64*b1 then max(.,0) gives 64*y, and
                # W2.T/64 in cwr cancels the scale at layer 2.
                if t < NT - 1:
                    nc.vector.tensor_scalar(out=y_sb[:], in0=psum_y[t][:],
                                            scalar1=cw_sb[:, 0:1],
                                            scalar2=0.0, op0=OP.add,
                                            op1=OP.max)
                    nc.tensor.matmul(y2_ps[:], cwr_sb[:], y_sb[:],
                                     start=True, stop=True)
                    nc.scalar.activation(out=y2_sb[:], in_=y2_ps[:],
                                         func=AF.Relu, bias=cw_sb[:, 1:2],
                                         scale=1.0)
                else:
                    # The last tile's relu/layer-2/relu2 sit on the exposed
                    # post-stream critical path: run each in halves on
                    # BOTH DVE and ACT so the stages overlap.
                    h0, h1 = slice(0, 256), slice(256, 512)
                    nc.vector.tensor_scalar(out=y_sb[:, h0],
                                            in0=psum_y[t][:, h0],
                                            scalar1=cw_sb[:, 0:1],
                                            scalar2=0.0, op0=OP.add,
                                            op1=OP.max)
                    nc.scalar.activation(out=y_sb[:, h1],
                                         in_=psum_y[t][:, h1], func=AF.Relu,
                                         bias=cw_sb[:, 0:1], scale=1.0)
                    nc.tensor.matmul(y2_ps[:, h0], cwr_sb[:], y_sb[:, h0],
                                     start=True, stop=True)
                    nc.tensor.matmul(y2_ps[:, h1], cwr_sb[:], y_sb[:, h1],
                                     start=True, stop=True)
                    nc.scalar.activation(out=y2_sb[:, h0], in_=y2_ps[:, h0],
                                         func=AF.Relu, bias=cw_sb[:, 1:2],
                                         scale=1.0)
                    nc.vector.tensor_scalar(out=y2_sb[:, h1],
                                            in0=y2_ps[:, h1],
                                            scalar1=cw_sb[:, 1:2],
                                            scalar2=0.0, op0=OP.add,
                                            op1=OP.max)
                # q|v head projections into one (128, 256) psum (q cols
                # 0:128, v cols 128:256), one 8-lane matmul per quarter at
                # partition offset 32*qq.
                qv_ps = ps_qv.tile([128, 256], f32, tag="qv",
                                   name=f"qv_ps{t}")
                for qq in range(NQ):
                    rhs = y2_sb[:, 128 * qq:128 * (qq + 1)]
                    nc.tensor.matmul(qv_ps[32 * qq:32 * qq + NH, 0:128],
                                     chd_sb[:, 0:NH], rhs,
                                     start=True, stop=True,
                                     tile_position=(0, 32 * qq))
                    nc.tensor.matmul(qv_ps[32 * qq:32 * qq + NH, 128:256],
                                     chd_sb[:, NH:2 * NH], rhs,
                                     start=True, stop=True,
                                     tile_position=(0, 32 * qq))
                if affine:
                    add_blk = ca_blk[:, 0:128]
                    bv_col = ca_blk[:, 128:129]
                else:
                    add_blk = ca_sb[:, 128 * t:128 * (t + 1)]
                    bv_col = ca_sb[:, NT * 128:NT * 128 + 1]
                # l' = q + B*n (affine: the lane-constant A is dropped here
                # and folded into the host-side merge) or q + full table
                l_sb = smallp.tile([128, 128], f32, tag="l", name=f"l_sb{t}")
                nc.vector.tensor_add(out=l_sb[:], in0=qv_ps[:, 0:128],
                                     in1=add_blk)
                e_sb = smallp.tile([128, 128], f32, tag="e", name=f"e_sb{t}")
                if affine:
                    # No on-chip max: l' = q + B*n is bounded (|q| <~ 10,
                    # |B*n| <= ~3), so exp(l' - 16) stays finite and the
                    # common e^{A+16-m} scale cancels in the host's f64
                    # W/Z merge (m col of stats is left unwritten; the host
                    # uses m = A + 16).  Removes the max reduce from the
                    # exposed chain, so exp starts right after the add.
                    nc.scalar.activation(out=e_sb[:], in_=l_sb[:],
                                         func=AF.Exp,
                                         bias=ca_blk[:, 129:130], scale=1.0,
                                         accum_out=stats_sb[:,
                                                            3 * t + 1:
                                                            3 * t + 2])
                else:
                    # stats[:, 0] = -max_n l   (valid on 32qq+h lanes)
                    nc.vector.tensor_reduce(out=stats_sb[:, 3 * t:3 * t + 1],
                                            in_=l_sb[:], axis=AX.X,
                                            op=OP.max, negate=True)
                    # e = exp(l - max); stats[:, 1] = Z = sum e
                    nc.scalar.activation(out=e_sb[:], in_=l_sb[:],
                                         func=AF.Exp,
                                         bias=stats_sb[:, 3 * t:3 * t + 1],
                                         scale=1.0,
                                         accum_out=stats_sb[:,
                                                            3 * t + 1:
                                                            3 * t + 2])
                # Fused (v + bv) * e with row-sum: stats[:, 2] = W = sum e*v
                ev_sb = smallp.tile([128, 128], f32, tag="ev",
                                    name=f"ev_sb{t}")
                nc.vector.scalar_tensor_tensor(
                    out=ev_sb[:], in0=qv_ps[:, 128:256], scalar=bv_col,
                    in1=e_sb[:], op0=OP.add, op1=OP.mult,
                    accum_out=stats_sb[:, 3 * t + 2:3 * t + 3])

            nc.sync.dma_start(out=st_d[:], in_=stats_sb[:])

    nc.finalize()
    return nc


def get_nc(affine):
    key = ("nc", bool(affine))
    if key not in _cache:
        _cache[key] = _build_nc(bool(affine))
    return _cache[key]


def make_core_inputs(x, mask, W1, b1, W2, b2, Wq, Wv, bv, pos_w, bias,
                     affine):
    """Host-side shard + transpose + fp8 quantization.  Returns list of 8
    in_maps."""
    import ml_dtypes
    e4 = ml_dtypes.float8_e4m3     # TRN FP8_EXP4: bias 7, max +-240, has inf

    # w1s[p, c, j, m] = 64 * W1[m, c*256 + j*128 + p], e4m3
    w1q = (W1.astype(np.float32) * np.float32(W1SCALE)).astype(e4)
    w1s = np.ascontiguousarray(
        w1q.reshape(MLP, KC2, 2, 128).transpose(3, 1, 2, 0))

    cwr = np.ascontiguousarray((W2.T / np.float32(W1SCALE)).astype(np.float32))
    chd = np.concatenate([Wq.T, Wv.T], axis=1).astype(ml_dtypes.bfloat16)
    cw = np.stack([b1.astype(np.float32) * np.float32(W1SCALE),
                   b2.astype(np.float32)], axis=1)  # (MLP, 2)

    pos = np.arange(S, dtype=np.float32)
    maskadd = np.where(mask == 0, np.float32(-1e9), np.float32(0.0))  # (B,S)

    in_maps = []
    for c in range(NCORES):
        sl = slice(c * S_SHARD, (c + 1) * S_SHARD)
        # xt[c2*128+p, j, tok]: feature f = c2*256 + j*128 + p,
        # tok = batch*512 + local seq
        xq = x[:, sl, :].astype(e4)                    # (B, 512, H)
        xf = (xq.transpose(2, 0, 1)                    # (H, B, 512)
                .reshape(KC2, 2, 128, TOK)             # (c2, j, p, tok)
                .transpose(0, 2, 1, 3))                # (c2, p, j, tok)
        xt = np.ascontiguousarray(xf).reshape(KC2 * 128, 2, TOK)
        im = {"xt": xt, "w1s": w1s, "cw": cw, "cwr": cwr,
              "chd": chd}
        if affine:
            # cab: [A | B] coefficient blocks (additive logit = A + B*n on
            # lanes 32qq+h, same for every tile -- mask is all ones), bv
            # block (B=0), ramp block (row0 = ones, row1 = iota).
            # A (the lane-constant pos_w*(s0+128qq) part) is folded into the
            # host merge, so the on-chip table is just B*n.
            cab = np.zeros((2, 4 * 128), dtype=np.float32)
            for qq in range(NQ):
                lanes = slice(32 * qq, 32 * qq + NH)
                cab[1, lanes] = pos_w
                cab[0, 128 + 32 * qq:128 + 32 * qq + NH] = bv
            cab[0, 256:384] = -16.0
            cab[0, 384:512] = 1.0
            cab[1, 384:512] = np.arange(128, dtype=np.float32)
            im["cab"] = cab
        else:
            # ca: per tile a (128, 128) block; lane 32qq+h: additive logit
            # for seq position qq*128+n.  Last col: bv on the same lanes.
            ca = np.zeros((128, NT * 128 + 1), dtype=np.float32)
            addv = (pos_w.astype(np.float32)[None, :, None]
                    * pos[sl][None, None, :]
                    + maskadd[:, None, sl])            # (B, NH, 512)
            addv = addv.reshape(NT, NH, NQ, 128)
            for t in range(NT):
                for qq in range(NQ):
                    ca[32 * qq:32 * qq + NH, 128 * t:128 * (t + 1)] = \
                        addv[t, :, qq, :]
            for qq in range(NQ):
                ca[32 * qq:32 * qq + NH, NT * 128] = bv
            im["ca"] = ca
        in_maps.append(im)
    return in_maps


def merge_stats(stats_all, bias, m_host=None):
    """stats_all: (NCORES, 128, NT*3); lane 32qq+h of col block 3t holds
    [-m, Z, W] for (core, quarter qq, batch t, head h) -> (B, 1) output.
    With m_host (affine path) the m col is unwritten on-chip and m is the
    host-known exp offset instead."""
    st = np.asarray(stats_all, dtype=np.float64).reshape(NCORES, NQ, 32,
                                                         NT, 3)
    st = st[:, :, 0:NH]                   # (C, NQ, NH, NT, 3) valid lanes
    if m_host is not None:
        m = np.broadcast_to(m_host[..., None], st.shape[:-1])
    else:
        m = -st[..., 0]                   # (C, NQ, NH, NT)
    Z = st[..., 1]
    W = st[..., 2]
    M = m.max(axis=(0, 1))                # (NH, NT)
    alpha = np.exp(m - M[None, None])
    Zg = (alpha * Z).sum(axis=(0, 1))     # (NH, NT)
    Wg = (alpha * W).sum(axis=(0, 1))
    out = (Wg / Zg).sum(axis=0)           # (NT,) = (B,)
    return (out[:, None] + np.float64(bias.reshape(1)[0])).astype(np.float32)


def kernel(x, mask, W1, b1, W2, b2, Wq, Wv, bv, pos_w, bias, _trace=False):
    from concourse.bass_utils import run_bass_kernel_spmd

    x = np.asarray(x, dtype=np.float32)
    mask = np.asarray(mask)
    # With an all-ones mask (this problem's setup) the additive logit is
    # affine in position and is generated on-chip; arbitrary masks fall
    # back to streaming the full table.
    affine = bool(np.all(mask != 0))
    in_maps = make_core_inputs(x, mask, *(np.asarray(a) for a in
                               (W1, b1, W2, b2, Wq, Wv, bv, pos_w, bias)),
                               affine)
    nc = get_nc(affine)
    res = run_bass_kernel_spmd(nc, in_maps, core_ids=list(range(NCORES)),
                               trace=_trace)
    stats_all = np.stack([r["stats"] for r in res.results])  # (C, 128, NT*3)
    m_host = None
    if affine:
        # exp offset used on-chip: A + 16 with A = pos_w[h]*(s0 + 128*qq)
        pw = np.asarray(pos_w, dtype=np.float64)
        s0 = np.arange(NCORES, dtype=np.float64)[:, None, None] * S_SHARD
        qq0 = np.arange(NQ, dtype=np.float64)[None, :, None] * 128
        m_host = pw[None, None, :] * (s0 + qq0) + 16.0   # (C, NQ, NH)
    out = merge_stats(stats_all, np.asarray(bias), m_host)
    if _trace:
        kernel.last_result = res
    return out


# revision 63
# speedup vs baseline: 1.2374x; 1.2374x over previous
"""Trainium2 Bass kernel for nn_AttentionProbe_80891414053184.

Math (reference):
    y  = relu(x @ W1.T + b1)            # (B,S,H) -> (B,S,128)
    y2 = relu(y @ W2.T + b2)            # (B,S,128)
    l  = y2 @ Wq.T + pos*pos_w  (+mask) # (B,S,8) logits
    p  = softmax(l, axis=S)
    v  = y2 @ Wv.T + bv
    out[b] = sum_{s,h} p*v + bias       # (B,1)

Strategy: sequence-parallel over 8 cores (512 positions x 4 batches = 2048
tokens per core).  Each core streams its x-shard quantized to fp8-e4m3
(half the HBM bytes of bf16; end-to-end rel-err vs the fp32 reference
~5e-3, HW-measured) CHUNK-MAJOR across all 4 token tiles, layer 1 as
DoubleRow fp8 matmuls (256-deep contraction per instruction).  Chunk-major
keeps PE duty during the stream at ~60%, under the package power-throttle
threshold -- tile-major variants that fill the PE get DVFS-clamped to 50%
and lose more than they gain (HW-measured).  The last two chunks land
per-tile so each tile's MLP tail + softmax stats start as soon as its
accumulation closes.  Per-tile partial softmax stats (-max, Z, W) are
emitted per (seq-quarter, head) lane; the host merges 8 cores x 4
quarters with the standard online-softmax combine into the (4,1) output.

fp8 scaling: W1 is pre-scaled by 64 on the host so its N(0, 1/4096)
entries land in e4m3's normal range (min normal 2^-6); the 64x is folded
back via b1*64 at the relu (relu commutes with positive scale) and W2/64
in the layer-2 weights.  x itself is N(0,1) -- quantized unscaled.

Stats packing: per tile, q and v head projections land in one (128, 256)
psum (q cols 0:128, v cols 128:256), one 8-lane matmul per seq quarter at
partition offset 32*qq via tile_position.  The softmax-stats chain then
runs on 128-column DVE/ACT ops (~3x shorter than a 512-column chain).
bf16 is used for y2 and the head weights: the 128-col moving operand
would hit f32r's 4x penalty below 256 columns.
"""

import numpy as np

# Problem dims (hardcoded per harness contract).
B, S, H = 4, 4096, 4096
MLP, NH = 128, 8
NCORES = 8
S_SHARD = S // NCORES        # 512 seq positions per core
TOK = B * S_SHARD            # 2048 tokens per core
NT = TOK // 512              # 4 token tiles of 512 (= one batch each)
NQ = 4                       # seq quarters per tile (128 cols each)
KC2 = H // 256               # 16 double-chunks (256-deep DoubleRow contraction)
GRP = 2                      # double-chunks per streaming x DMA (1 MB fp8)
NGRP = KC2 // GRP            # 8 groups; the last arrives per (tile, chunk)
W1SCALE = 64.0               # fp8 pre-scale for W1 (power of 2, exact)

_cache = {}


def _build_nc(affine):
    """affine=True: the additive logit (pos_w*pos, mask all-ones) is
    generated on-chip by a rank-2 matmul pre-filling the q/v psum -- saves
    the 262 KB ca table from the HBM stream and two DVE chain steps.
    affine=False: general-mask path, ca table streamed from HBM."""
    import concourse.mybir as mybir
    import concourse.tile as tile
    from concourse import bacc

    f32 = mybir.dt.float32
    f32r = mybir.dt.float32r
    fp8 = mybir.dt.float8e4
    bf16 = mybir.dt.bfloat16
    DR = mybir.MatmulPerfMode.DoubleRow

    # Bacc (not bare Bass): its finalize() runs move_matmul_waits_to_ldweights
    # and generate_event_semaphores, which split multi-sem waits to satisfy
    # TRN2's one-wait-per-instruction encoding limit.
    nc = bacc.Bacc()
    # xt row r = c*128 + p holds [j=0 | j=1] token rows for feature
    # f = c*256 + j*128 + p -- the DoubleRow kxn layout with the two k-tile
    # planes exactly 2048 B apart in SBUF (other spacings hit SBUF
    # conflicts that halve the double-pumped PE read rate, HW-measured).
    xt_d = nc.dram_tensor("xt", [KC2 * 128, 2, TOK], fp8,
                          kind="ExternalInput")
    # w1s[p, c, j, m] = 64 * W1[m, c*256 + j*128 + p]
    w1_d = nc.dram_tensor("w1s", [128, KC2, 2, MLP], fp8, kind="ExternalInput")
    # cwr: W2.T/64 f32r -- the layer-2 matmul at the fast PE rate
    cwr_d = nc.dram_tensor("cwr", [MLP, MLP], f32r, kind="ExternalInput")
    # chd: [Wq.T | Wv.T] bf16
    chd_d = nc.dram_tensor("chd", [MLP, 2 * NH], bf16, kind="ExternalInput")
    # cw: [64*b1 | b2] per-partition scalar columns
    cw_d = nc.dram_tensor("cw", [MLP, 2], f32, kind="ExternalInput")
    if affine:
        # cab row 0/1 = (A, B) coefficient column blocks: the additive
        # logit A+B*n on lanes 32*qq+h (identical for every tile -- mask is
        # all ones), then a bv block (B=0), then the ramp block (row0 =
        # ones, row1 = iota) used as the moving operand.
        CBV = 128                    # bv block col offset
        CM16 = 256                   # constant -16 block (exp offset)
        CRMP = 384                   # ramp block col offset
        cab_d = nc.dram_tensor("cab", [2, CRMP + 128], f32,
                               kind="ExternalInput")
    else:
        # ca: per tile a (128, 128) additive-logit block (pos_w*pos + mask)
        # on lanes 32*qq+h; final col = bv on the same lanes.
        ca_d = nc.dram_tensor("ca", [128, NT * 128 + 1], f32,
                              kind="ExternalInput")
    # stats: per tile 3 cols [-m | Z | W]; lanes 32*qq+h are valid.
    st_d = nc.dram_tensor("stats", [128, NT * 3], f32, kind="ExternalOutput")

    AF = mybir.ActivationFunctionType
    AX = mybir.AxisListType
    OP = mybir.AluOpType

    with tile.TileContext(nc) as tc:
        with (
            tc.tile_pool(name="const", bufs=1) as const,
            tc.tile_pool(name="yp", bufs=2) as yp,
            tc.tile_pool(name="y2p", bufs=2) as y2p,
            tc.tile_pool(name="smallp", bufs=2) as smallp,
            tc.tile_pool(name="statsp", bufs=1) as statsp,
            tc.tile_pool(name="ps_y", bufs=4, space="PSUM") as ps_y,
            tc.tile_pool(name="ps_y2", bufs=2, space="PSUM") as ps_y2,
            tc.tile_pool(name="ps_qv", bufs=2, space="PSUM") as ps_qv,
        ):
            # The full fp8 x-shard lives in SBUF (64 KB/partition): no slot
            # recycling, so the stream DMAs carry no WAR deps on the PE and
            # need no escort ops.  Issue every x DMA up front on the gpsimd
            # (SWDGE) queue; Q7 descriptor emission (~1.2 us each) stays
            # ahead of the ~3 us per-group transfer time.
            x_sb = const.tile([128, KC2, 2, TOK], fp8)
            # Group 0 rides the sync (HWDGE) ring: it starts right at the
            # init-barrier exit (~0.9 us before the gpsimd Q7 wakes), and
            # the SDMA engines drain both rings concurrently, so the whole
            # stream finishes earlier.
            for g in range(NGRP - 1):
                eng = nc.sync if g == 0 else nc.gpsimd
                eng.dma_start(
                    out=x_sb[:, g * GRP:(g + 1) * GRP, :, :],
                    in_=xt_d[g * GRP * 128:(g + 1) * GRP * 128, :, :].rearrange(
                        "(a p) j n -> p a j n", p=128))
            # Chunk 14 for all tiles rides one half-group; then a single
            # 128 KB chunk-15 closer per tile, so tile t's accumulation
            # closes (and its tail starts) with minimal end-of-stream
            # serialization.
            nc.gpsimd.dma_start(
                out=x_sb[:, 14:15, :, :],
                in_=xt_d[14 * 128:15 * 128, :, :].rearrange(
                    "(a p) j n -> p a j n", p=128))
            for t in range(NT):
                nc.gpsimd.dma_start(
                    out=x_sb[:, 15, :, t * 512:(t + 1) * 512],
                    in_=xt_d[15 * 128:16 * 128, :,
                             t * 512:(t + 1) * 512].rearrange(
                                 "(p) j n -> p j n", p=128))

            # Const loads on the sync (HWDGE) queue -- separate path from the
            # x stream.  w1 chunk 0 is split out so the first matmul gates on
            # a 32 KB transfer, not the full 512 KB.
            w1_sb = const.tile([128, KC2, 2, MLP], fp8)
            nc.sync.dma_start(out=w1_sb[:, 0:1, :, :], in_=w1_d[:, 0:1, :, :])
            nc.sync.dma_start(out=w1_sb[:, 1:KC2, :, :], in_=w1_d[:, 1:KC2, :, :])
            if affine:
                cab_sb = const.tile([2, CRMP + 128], f32)
                nc.sync.dma_start(out=cab_sb[:], in_=cab_d[:])
            else:
                ca_sb = const.tile([128, NT * 128 + 1], f32)
                nc.sync.dma_start(out=ca_sb[:], in_=ca_d[:])
            cw_sb = const.tile([MLP, 2], f32)
            nc.sync.dma_start(out=cw_sb[:], in_=cw_d[:])
            cwr_sb = const.tile([MLP, MLP], f32r)
            nc.sync.dma_start(out=cwr_sb[:], in_=cwr_d[:])
            chd_sb = const.tile([MLP, 2 * NH], bf16)
            nc.sync.dma_start(out=chd_sb[:], in_=chd_d[:])

            stats_sb = statsp.tile([128, NT * 3], f32)

            # --- Warmup / staging: each engine observes every const-DMA lane
            # once, so steady-state instructions carry at most one new wait
            # (fewer split-events from Bacc's generate_event_semaphores).
            # Only the w1 warmup gates the k-loop; the other warms run later
            # so the first real matmul waits on nothing but w1-chunk0 +
            # x-group0.
            warm_ps = ps_y2.tile([128, 512], f32, tag="y2", name="warm_ps")
            warm_pe_last = nc.tensor.matmul(warm_ps[:, 0:NH],
                                            w1_sb[:, 0, :, :],
                                            w1_sb[:, 0, :, 0:NH],
                                            start=True, stop=True,
                                            perf_mode=DR)

            # Layer 1, chunk-major over the streaming groups: yT[t]
            # (128, 512) += (64*W1T)_c.T @ xT_c, DoubleRow fp8 (256-deep
            # contraction per matmul), k-accumulated over 16 double-chunks.
            psum_y = [ps_y.tile([128, 512], f32, tag="y", name=f"y_ps{t}")
                      for t in range(NT)]
            first = True
            for c in range(GRP * (NGRP - 1)):
                for t in range(NT):
                    mm = nc.tensor.matmul(
                        psum_y[t][:],
                        w1_sb[:, c, :, :],
                        x_sb[:, c, :, t * 512:(t + 1) * 512],
                        start=(c == 0),
                        stop=False,
                        perf_mode=DR,
                    )
                    if first:
                        first = False
                        tile.add_dep_helper(mm.ins, warm_pe_last.ins,
                                            sync=False,
                                            reason="warmups before first mm")

            # cw/ca/cwr/chd lane warmups (before their first consumers in
            # tile 0's tail)
            warm_ps2 = ps_y2.tile([128, 512], f32, tag="y2", name="warm_ps2")
            nc.tensor.matmul(warm_ps2[0:NH, 0:NH], cwr_sb[:, 0:NH],
                             cwr_sb[:, 0:NH], start=True, stop=True)
            nc.tensor.matmul(warm_ps2[0:2 * NH, NH:2 * NH], chd_sb[:],
                             chd_sb[:, 0:NH], start=True, stop=True)
            warm_act = const.tile([MLP, 1], f32)
            nc.scalar.copy(out=warm_act[:], in_=cw_sb[:, 1:2])
            warm_dve = const.tile([128, 1], f32)
            if affine:
                # Build the (tile-invariant) additive-logit block A + B*n
                # and the bv column once, on-chip: two rank-2 matmuls into
                # psum, one DVE copy to SBUF.  Replaces the 262 KB ca table
                # in the HBM stream.
                ab_ps = ps_y2.tile([128, 512], f32, tag="y2", name="ab_ps")
                nc.tensor.matmul(ab_ps[:, 0:128], cab_sb[:, 0:128],
                                 cab_sb[:, CRMP:CRMP + 128],
                                 start=True, stop=True)
                nc.tensor.matmul(ab_ps[:, 128:129],
                                 cab_sb[:, CBV:CBV + 128],
                                 cab_sb[:, CRMP:CRMP + 1],
                                 start=True, stop=True)
                nc.tensor.matmul(ab_ps[:, 129:130],
                                 cab_sb[:, CM16:CM16 + 128],
                                 cab_sb[:, CRMP:CRMP + 1],
                                 start=True, stop=True)
                ca_blk = const.tile([128, 130], f32)
                nc.vector.tensor_copy(out=ca_blk[:], in_=ab_ps[:, 0:130])
                nc.vector.tensor_copy(out=warm_dve[:], in_=cw_sb[:, 0:1])
            else:
                warm_act8 = const.tile([128, 1], f32)
                nc.scalar.copy(out=warm_act8[:],
                               in_=ca_sb[:, NT * 128:NT * 128 + 1])
                nc.vector.tensor_copy(out=warm_dve[:], in_=ca_sb[:, 0:1])

            # Per tile: close the accumulation with its own last chunks,
            # then run the MLP tail + softmax stats for that tile while the
            # next tile's last chunks land.
            for t in range(NT):
                for c in range(GRP * (NGRP - 1), KC2):
                    nc.tensor.matmul(
                        psum_y[t][:],
                        w1_sb[:, c, :, :],
                        x_sb[:, c, :, t * 512:(t + 1) * 512],
                        start=False,
                        stop=(c == KC2 - 1),
                        perf_mode=DR,
                    )

                y_sb = yp.tile([128, 512], f32r, tag="ysb", name=f"y_sb{t}")
                y2_ps = ps_y2.tile([128, 512], f32, tag="y2", name=f"y2_ps{t}")
                y2_sb = y2p.tile([128, 512], bf16, tag="y2sb",
                                 name=f"y2_sb{t}")
                # relu on DVE (add+max) keeps ACT free for relu2/exp.  psum
                # holds 64*(x@W1.T); +64*b1 then max(.,0) gives 64*y, and
                # W2.T/64 in cwr cancels the scale at layer 2.
                if t < NT - 1:
                    nc.vector.tensor_scalar(out=y_sb[:], in0=psum_y[t][:],
                                            scalar1=cw_sb[:, 0:1],
                                            scalar2=0.0, op0=OP.add,
                                            op1=OP.max)
                    nc.tensor.matmul(y2_ps[:], cwr_sb[:], y_sb[:],
                                     start=True, stop=True)
                    nc.scalar.activation(out=y2_sb[:], in_=y2_ps[:],
                                         func=AF.Relu, bias=cw_sb[:, 1:2],
                                         scale=1.0)
                else:
                    # The last tile's relu/layer-2/relu2 sit on the exposed
                    # post-stream critical path: run each in halves on
                    # BOTH DVE and ACT so the stages overlap.
                    h0, h1 = slice(0, 256), slice(256, 512)
                    nc.vector.tensor_scalar(out=y_sb[:, h0],
                                            in0=psum_y[t][:, h0],
                                            scalar1=cw_sb[:, 0:1],
                                            scalar2=0.0, op0=OP.add,
                                            op1=OP.max)
                    nc.scalar.activation(out=y_sb[:, h1],
                                         in_=psum_y[t][:, h1], func=AF.Relu,
                                         bias=cw_sb[:, 0:1], scale=1.0)
                    nc.tensor.matmul(y2_ps[:, h0], cwr_sb[:], y_sb[:, h0],
                                     start=True, stop=True)
                    nc.tensor.matmul(y2_ps[:, h1], cwr_sb[:], y_sb[:, h1],
                                     start=True, stop=True)
                    nc.scalar.activation(out=y2_sb[:, h0], in_=y2_ps[:, h0],
                                         func=AF.Relu, bias=cw_sb[:, 1:2],
                                         scale=1.0)
                    nc.vector.tensor_scalar(out=y2_sb[:, h1],
                                            in0=y2_ps[:, h1],
                                            scalar1=cw_sb[:, 1:2],
                                            scalar2=0.0, op0=OP.add,
                                            op1=OP.max)
                # q|v head projections into one (128, 256) psum (q cols
                # 0:128, v cols 128:256), one 8-lane matmul per quarter at
                # partition offset 32*qq.
                qv_ps = ps_qv.tile([128, 256], f32, tag="qv",
                                   name=f"qv_ps{t}")
                for qq in range(NQ):
                    rhs = y2_sb[:, 128 * qq:128 * (qq + 1)]
                    nc.tensor.matmul(qv_ps[32 * qq:32 * qq + NH, 0:128],
                                     chd_sb[:, 0:NH], rhs,
                                     start=True, stop=True,
                                     tile_position=(0, 32 * qq))
                    nc.tensor.matmul(qv_ps[32 * qq:32 * qq + NH, 128:256],
                                     chd_sb[:, NH:2 * NH], rhs,
                                     start=True, stop=True,
                                     tile_position=(0, 32 * qq))
                if affine:
                    add_blk = ca_blk[:, 0:128]
                    bv_col = ca_blk[:, 128:129]
                else:
                    add_blk = ca_sb[:, 128 * t:128 * (t + 1)]
                    bv_col = ca_sb[:, NT * 128:NT * 128 + 1]
                # l' = q + B*n (affine: the lane-constant A is dropped here
                # and folded into the host-side merge) or q + full table
                l_sb = smallp.tile([128, 128], f32, tag="l", name=f"l_sb{t}")
                nc.vector.tensor_add(out=l_sb[:], in0=qv_ps[:, 0:128],
                                     in1=add_blk)
                e_sb = smallp.tile([128, 128], f32, tag="e", name=f"e_sb{t}")
                if affine:
                    # No on-chip max: l' = q + B*n is bounded (|q| <~ 10,
                    # |B*n| <= ~3), so exp(l' - 16) stays finite and the
                    # common e^{A+16-m} scale cancels in the host's f64
                    # W/Z merge (m col of stats is left unwritten; the host
                    # uses m = A + 16).  Removes the max reduce from the
                    # exposed chain, so exp starts right after the add.
                    nc.scalar.activation(out=e_sb[:], in_=l_sb[:],
                                         func=AF.Exp,
                                         bias=ca_blk[:, 129:130], scale=1.0,
                                         accum_out=stats_sb[:,
                                                            3 * t + 1:
                                                            3 * t + 2])
                else:
                    # stats[:, 0] = -max_n l   (valid on 32qq+h lanes)
                    nc.vector.tensor_reduce(out=stats_sb[:, 3 * t:3 * t + 1],
                                            in_=l_sb[:], axis=AX.X,
                                            op=OP.max, negate=True)
                    # e = exp(l - max); stats[:, 1] = Z = sum e
                    nc.scalar.activation(out=e_sb[:], in_=l_sb[:],
                                         func=AF.Exp,
                                         bias=stats_sb[:, 3 * t:3 * t + 1],
                                         scale=1.0,
                                         accum_out=stats_sb[:,
                                                            3 * t + 1:
                                                            3 * t + 2])
                # Fused (v + bv) * e with row-sum: stats[:, 2] = W = sum e*v
                ev_sb = smallp.tile([128, 128], f32, tag="ev",
                                    name=f"ev_sb{t}")
                nc.vector.scalar_tensor_tensor(
                    out=ev_sb[:], in0=qv_ps[:, 128:256], scalar=bv_col,
                    in1=e_sb[:], op0=OP.add, op1=OP.mult,
                    accum_out=stats_sb[:, 3 * t + 2:3 * t + 3])

            nc.sync.dma_start(out=st_d[:], in_=stats_sb[:])

    nc.finalize()
    return nc


def get_nc(affine):
    key = ("nc", bool(affine))
    if key not in _cache:
        _cache[key] = _build_nc(bool(affine))
    return _cache[key]


def make_core_inputs(x, mask, W1, b1, W2, b2, Wq, Wv, bv, pos_w, bias,
                     affine):
    """Host-side shard + transpose + fp8 quantization.  Returns list of 8
    in_maps."""
    import ml_dtypes
    e4 = ml_dtypes.float8_e4m3     # TRN FP8_EXP4: bias 7, max +-240, has inf

    # w1s[p, c, j, m] = 64 * W1[m, c*256 + j*128 + p], e4m3
    w1q = (W1.astype(np.float32) * np.float32(W1SCALE)).astype(e4)
    w1s = np.ascontiguousarray(
        w1q.reshape(MLP, KC2, 2, 128).transpose(3, 1, 2, 0))

    cwr = np.ascontiguousarray((W2.T / np.float32(W1SCALE)).astype(np.float32))
    chd = np.concatenate([Wq.T, Wv.T], axis=1).astype(ml_dtypes.bfloat16)
    cw = np.stack([b1.astype(np.float32) * np.float32(W1SCALE),
                   b2.astype(np.float32)], axis=1)  # (MLP, 2)

    pos = np.arange(S, dtype=np.float32)
    maskadd = np.where(mask == 0, np.float32(-1e9), np.float32(0.0))  # (B,S)

    in_maps = []
    for c in range(NCORES):
        sl = slice(c * S_SHARD, (c + 1) * S_SHARD)
        # xt[c2*128+p, j, tok]: feature f = c2*256 + j*128 + p,
        # tok = batch*512 + local seq
        xq = x[:, sl, :].astype(e4)                    # (B, 512, H)
        xf = (xq.transpose(2, 0, 1)                    # (H, B, 512)
                .reshape(KC2, 2, 128, TOK)             # (c2, j, p, tok)
                .transpose(0, 2, 1, 3))                # (c2, p, j, tok)
        xt = np.ascontiguousarray(xf).reshape(KC2 * 128, 2, TOK)
        im = {"xt": xt, "w1s": w1s, "cw": cw, "cwr": cwr,
              "chd": chd}
        if affine:
            # cab: [A | B] coefficient blocks (additive logit = A + B*n on
            # lanes 32qq+h, same for every tile -- mask is all ones), bv
            # block (B=0), ramp block (row0 = ones, row1 = iota).
            # A (the lane-constant pos_w*(s0+128qq) part) is folded into the
            # host merge, so the on-chip table is just B*n.
            cab = np.zeros((2, 4 * 128), dtype=np.float32)
            for qq in range(NQ):
                lanes = slice(32 * qq, 32 * qq + NH)
                cab[1, lanes] = pos_w
                cab[0, 128 + 32 * qq:128 + 32 * qq + NH] = bv
            cab[0, 256:384] = -16.0
            cab[0, 384:512] = 1.0
            cab[1, 384:512] = np.arange(128, dtype=np.float32)
            im["cab"] = cab
        else:
            # ca: per tile a (128, 128) block; lane 32qq+h: additive logit
            # for seq position qq*128+n.  Last col: bv on the same lanes.
            ca = np.zeros((128, NT * 128 + 1), dtype=np.float32)
            addv = (pos_w.astype(np.float32)[None, :, None]
                    * pos[sl][None, None, :]
                    + maskadd[:, None, sl])            # (B, NH, 512)
            addv = addv.reshape(NT, NH, NQ, 128)
            for t in range(NT):
                for qq in range(NQ):
                    ca[32 * qq:32 * qq + NH, 128 * t:128 * (t + 1)] = \
                        addv[t, :, qq, :]
            for qq in range(NQ):
                ca[32 * qq:32 * qq + NH, NT * 128] = bv
            im["ca"] = ca
        in_maps.append(im)
    return in_maps


def merge_stats(stats_all, bias, m_host=None):
    """stats_all: (NCORES, 128, NT*3); lane 32qq+h of col block 3t holds
    [-m, Z, W] for (core, quarter qq, batch t, head h) -> (B, 1) output.
    With m_host (affine path) the m col is unwritten on-chip and m is the
    host-known exp offset instead."""
    st = np.asarray(stats_all, dtype=np.float64).reshape(NCORES, NQ, 32,
                                                         NT, 3)
    st = st[:, :, 0:NH]                   # (C, NQ, NH, NT, 3) valid lanes
    if m_host is not None:
        m = np.broadcast_to(m_host[..., None], st.shape[:-1])
    else:
        m = -st[..., 0]                   # (C, NQ, NH, NT)
    Z = st[..., 1]
    W = st[..., 2]
    M = m.max(axis=(0, 1))                # (NH, NT)
    alpha = np.exp(m - M[None, None])
    Zg = (alpha * Z).sum(axis=(0, 1))     # (NH, NT)
    Wg = (alpha * W).sum(axis=(0, 1))
    out = (Wg / Zg).sum(axis=0)           # (NT,) = (B,)
    return (out[:, None] + np.float64(bias.reshape(1)[0])).astype(np.float32)


def kernel(x, mask, W1, b1, W2, b2, Wq, Wv, bv, pos_w, bias, _trace=False):
    from concourse.bass_utils import run_bass_kernel_spmd

    x = np.asarray(x, dtype=np.float32)
    mask = np.asarray(mask)
    # With an all-ones mask (this problem's setup) the additive logit is
    # affine in position and is generated on-chip; arbitrary masks fall
    # back to streaming the full table.
    affine = bool(np.all(mask != 0))
    in_maps = make_core_inputs(x, mask, *(np.asarray(a) for a in
                               (W1, b1, W2, b2, Wq, Wv, bv, pos_w, bias)),
                               affine)
    nc = get_nc(affine)
    res = run_bass_kernel_spmd(nc, in_maps, core_ids=list(range(NCORES)),
                               trace=_trace)
    stats_all = np.stack([r["stats"] for r in res.results])  # (C, 128, NT*3)
    m_host = None
    if affine:
        # exp offset used on-chip: A + 16 with A = pos_w[h]*(s0 + 128*qq)
        pw = np.asarray(pos_w, dtype=np.float64)
        s0 = np.arange(NCORES, dtype=np.float64)[:, None, None] * S_SHARD
        qq0 = np.arange(NQ, dtype=np.float64)[None, :, None] * 128
        m_host = pw[None, None, :] * (s0 + qq0) + 16.0   # (C, NQ, NH)
    out = merge_stats(stats_all, np.asarray(bias), m_host)
    if _trace:
        kernel.last_result = res
    return out
